# revision 47
# baseline (speedup 1.0000x reference)
"""Trainium2 Bass kernel for ContextQueryAttention (BiDAF-style).

Math (per batch):
    S[n,m] = c@w0 [n] + (q@w1 + bias)[m] + sum_d c[n,d]*wm[d]*q[m,d]
    S_  = softmax_m(S + MASK*(1-q_mask))          # row softmax
    S_T = softmax_n(S + MASK*(1-c_mask)).T        # col softmax, transposed
    c2q = S_ @ q ;  q2c = S_ @ (S_T @ c)
    out = [c | c2q | c*c2q | c*q2c]

Factorization: softmax shift-invariance lets the n-dependent terms ride
inside one exp.  With
    G'[n,m] = exp(sub2[n,m] + sub0[n] + 30*(c_mask[n]-1))
    Bq[m]   = exp(sub1[m] + bias[m]) * q_mask[m]
the two softmaxes reduce to
    S_[n,m]  = G'[n,m]*Bq[m] / (G' @ Bq)[n]     (exp(sub0)+mask cancel per row)
    S_T[m,n] = G'[n,m] / (G'^T @ 1)[m]          (Bq-terms cancel per column)
so no sub0 reduction, no c-side mask multiply, and the S_T@c matmul can
consume c directly with an appended ones column for the denominator.
sub0[n] (= c@w0, a [1,N] row) and the c_mask bias are added into the S^T
PSUM accumulation as K=1 rank-1 matmul updates before the exp.
30*(c_mask-1) is precomputed on host (exp(-30)~1e-13 is negligible, and
row-constant shifts cancel in the row softmax exactly as in the reference).

I/O and matmul operands are bf16 (inputs downcast and outputs upcast on
host): halves HBM traffic and engine element counts; PSUM accumulation
stays f32, per-partition scalars (bq, rsi, csi) stay f32.  The c output
quarter is stored straight from the loaded c tile (no compute).

Sharding: data-parallel over batch, 8 batches per core on 8 cores.
Pipeline: two batches deep — prep(b+1) is fully emitted before out(b)
begins, and prep(b+2) chunks are interleaved one at a time between
out-tile ops so no in-order engine queue parks ready out-work behind
prep-work, and the DMA engines stream stores continuously.
"""

import sys

if "/opt/trn_rl_repo" not in sys.path:
    sys.path.insert(0, "/opt/trn_rl_repo")

import numpy as np

import concourse.mybir as mybir
import concourse.tile as tile
from concourse import bacc
from concourse.bass_utils import run_bass_kernel_spmd
from concourse.masks import make_identity

B, N, M, D = 64, 1024, 128, 256
NCORES = 8
BPC = B // NCORES  # batches per core
NT = N // 128      # n-tiles per batch
DT = D // 128      # d-tiles

F32 = mybir.dt.float32
BF16 = mybir.dt.bfloat16
I32 = mybir.dt.int32
EXP = mybir.ActivationFunctionType.Exp


def _build(bpc: int = BPC, pb_bufs: int = 4, po_bufs: int = 6, big2_bufs: int = 2, tp_bufs: int = 2, big_bufs: int = 2):
    nc = bacc.Bacc(trn_type="TRN2")

    c_d = nc.dram_tensor("c", [bpc, N, D], BF16, kind="ExternalInput")
    q_d = nc.dram_tensor("q", [bpc, M, D], BF16, kind="ExternalInput")
    cmb_d = nc.dram_tensor("cmb", [bpc, N], F32, kind="ExternalInput")
    qm_d = nc.dram_tensor("q_mask", [bpc, M], I32, kind="ExternalInput")
    w0_d = nc.dram_tensor("w0", [D, 1], F32, kind="ExternalInput")
    w1_d = nc.dram_tensor("w1", [D, 1], F32, kind="ExternalInput")
    wm_d = nc.dram_tensor("wm", [D], F32, kind="ExternalInput")
    bias_d = nc.dram_tensor("bias", [M], F32, kind="ExternalInput")
    out_d = nc.dram_tensor("out", [bpc, N, 4 * D], BF16, kind="ExternalOutput")

    with tile.TileContext(nc) as tc:
        with (
            tc.tile_pool(name="glob", bufs=1) as gp,
            tc.tile_pool(name="pb", bufs=pb_bufs) as pb,
            tc.tile_pool(name="pscr", bufs=1) as pscr,
            tc.tile_pool(name="po", bufs=po_bufs) as po,
            tc.tile_pool(name="ps_tr", bufs=tp_bufs, space="PSUM") as ps_tr,
            tc.tile_pool(name="ps_big", bufs=big_bufs, space="PSUM") as ps_big,
            tc.tile_pool(name="ps_cq", bufs=big2_bufs, space="PSUM") as ps_cq,
        ):
            # ---- globals ----
            ident = gp.tile([128, 128], F32)
            make_identity(nc, ident)
            w1b = gp.tile([128, D], F32)
            nc.sync.dma_start(out=w1b, in_=w1_d[:, 0].partition_broadcast(128))
            w0c = gp.tile([128, DT], F32)
            nc.sync.dma_start(out=w0c, in_=w0_d[:, 0].rearrange("(j p) -> p j", p=128))
            wm_sb = gp.tile([128, DT], F32)
            nc.sync.dma_start(out=wm_sb, in_=wm_d[:].rearrange("(j p) -> p j", p=128))
            bias_sb = gp.tile([128, 1], F32)
            nc.sync.dma_start(out=bias_sb, in_=bias_d[:].rearrange("(o p) -> p o", p=128))
            zeros8 = gp.tile([128, NT], F32)
            nc.vector.memset(zeros8, 0.0)
            ones_r = gp.tile([1, 128], BF16)
            nc.vector.memset(ones_r, 1.0)
            w0r = gp.tile([128, DT], BF16)
            nc.vector.tensor_copy(w0r, w0c)
            identb = gp.tile([128, 128], BF16)
            nc.vector.tensor_copy(identb, ident)

            def load_stage(b):
                """Input DMAs only for batch b (no compute engines)."""
                st = {}
                c_n = pb.tile([128, NT, D + 2], BF16, tag="c_n")
                nc.sync.dma_start(
                    out=c_n[:, :, 0:D], in_=c_d[b].rearrange("(i p) d -> p i d", p=128)
                )
                qb_t = pb.tile([128, D], BF16, tag="qb_t")
                nc.sync.dma_start(out=qb_t, in_=q_d[b, :, :])
                qm_t = pb.tile([128, 1], I32, tag="qm_t")
                nc.sync.dma_start(
                    out=qm_t, in_=qm_d[b, :].rearrange("(o p) -> p o", p=128)
                )
                cmb_t = pb.tile([1, N], F32, tag="cmb_t")
                nc.sync.dma_start(
                    out=cmb_t, in_=cmb_d[b, :].rearrange("(o n) -> o n", o=1)
                )
                st["c_n"] = c_n
                st["qb_t"] = qb_t
                st["qm_t"] = qm_t
                st["cmb_t"] = cmb_t
                return st

            def prep_chunks(b, st):
                """Prep for batch b as a list of emission closures, so the
                pipeline can interleave them with out-tiles of batch b-1."""
                c_n, qb_t, qm_t, cmb_t = st["c_n"], st["qb_t"], st["qm_t"], st["cmb_t"]
                h = {}

                def ck_qside():
                    # ones column for the S_T@c denominator (+ zero pad:
                    # fp32r matmuls want an even column count)
                    nc.gpsimd.memset(c_n[:, :, D : D + 1], 1.0)
                    nc.gpsimd.memset(c_n[:, :, D + 1 : D + 2], 0.0)
                    mqf = pb.tile([128, 1], F32, tag="mqf")
                    nc.gpsimd.tensor_copy(mqf, qm_t)
                    scrq = pscr.tile([128, D], F32, tag="scrq")
                    sub1 = pb.tile([128, 1], F32, tag="sub1")
                    nc.vector.tensor_mul(scrq, qb_t, w1b)
                    nc.vector.reduce_sum(out=sub1, in_=scrq, axis=mybir.AxisListType.X)
                    bq0 = pb.tile([128, 1], F32, tag="bq0")
                    nc.scalar.activation(bq0, sub1, EXP, bias=bias_sb, scale=1.0)
                    bq = pb.tile([128, 1], F32, tag="bq")
                    nc.vector.tensor_mul(bq, bq0, mqf)
                    h["bq"] = bq

                def ck_cstore():
                    # the [c | ...] output quarter is the input verbatim:
                    # store it straight from SBUF, one descriptor per batch.
                    # Emitted as a chunk (not with the loads) so it never
                    # parks ready out-tile stores behind its c-load wait.
                    nc.sync.dma_start(
                        out=out_d[b, :, 0:D].rearrange("(i p) d -> p i d", p=128),
                        in_=c_n[:, :, 0:D],
                    )

                def ck_qmat():
                    bq = h["bq"]
                    qwmT = pb.tile([128, DT, 128], BF16, tag="qwmT")
                    tpq = ps_tr.tile([128, 256], BF16, tag="tp")
                    for j in range(DT):
                        nc.tensor.transpose(
                            tpq[:, 128 * j : 128 * (j + 1)],
                            qb_t[:, 128 * j : 128 * (j + 1)],
                            identb,
                        )
                        nc.vector.tensor_scalar_mul(
                            out=qwmT[:, j, :],
                            in0=tpq[:, 128 * j : 128 * (j + 1)],
                            scalar1=wm_sb[:, j : j + 1],
                        )
                    qBx = pb.tile([128, D + 2], BF16, tag="qBx")
                    nc.gpsimd.tensor_scalar_mul(out=qBx[:, 0:D], in0=qb_t, scalar1=bq)
                    nc.gpsimd.tensor_copy(qBx[:, D : D + 2], zeros8[:, 0:2])
                    nc.gpsimd.tensor_copy(qBx[:, D : D + 1], bq)
                    h["qwmT"] = qwmT
                    st["qBx"] = qBx

                def ck_cT0():
                    h["cT"] = pb.tile([128, DT, N], BF16, tag="cT", name="cT")

                def mk_cT(ip):
                    def ck():
                        cT = h["cT"]
                        for j in range(DT):
                            tp2 = ps_tr.tile([128, 512], BF16, tag="tp")
                            for u in range(4):
                                nc.tensor.transpose(
                                    tp2[:, 128 * u : 128 * (u + 1)],
                                    c_n[:, ip + u, 128 * j : 128 * (j + 1)],
                                    identb,
                                )
                            nc.scalar.copy(cT[:, j, 128 * ip : 128 * (ip + 4)], tp2)
                    return ck

                def ck_gt0():
                    h["GT"] = pb.tile([128, N], BF16, tag="GT", name="GT")
                    h["srow"] = pb.tile([1, N], BF16, tag="srow", name="srow")
                    st["GT"] = h["GT"]

                def mk_st(hh):
                    def ck():
                        cT, qwmT, GT, srow = h["cT"], h["qwmT"], h["GT"], h["srow"]
                        hs = slice(512 * hh, 512 * (hh + 1))
                        srp = ps_big.tile([1, 512], F32, tag="big")
                        for j in range(DT):
                            nc.tensor.matmul(
                                srp, w0r[:, j : j + 1], cT[:, j, hs],
                                start=(j == 0), stop=(j == DT - 1),
                            )
                        # srow = (c@w0) + 30*(c_mask-1); DVE add rounds bf16
                        nc.vector.tensor_add(srow[0:1, hs], srp, cmb_t[0:1, hs])
                        stp = ps_big.tile([128, 512], F32, tag="big")
                        for j in range(DT):
                            nc.tensor.matmul(
                                stp, qwmT[:, j, :], cT[:, j, hs],
                                start=(j == 0), stop=False,
                            )
                        nc.tensor.matmul(
                            stp, ones_r, srow[0:1, hs], start=False, stop=True
                        )
                        nc.scalar.activation(GT[:, hs], stp, EXP)
                    return ck

                def ck_gn0():
                    h["Gn"] = pb.tile([128, NT, 128], BF16, tag="Gn", name="Gn")

                def mk_gn(ip):
                    def ck():
                        GT, Gn = h["GT"], h["Gn"]
                        tp2 = ps_tr.tile([128, 512], BF16, tag="tp")
                        for u in range(4):
                            nc.tensor.transpose(
                                tp2[:, 128 * u : 128 * (u + 1)],
                                GT[:, 128 * (ip + u) : 128 * (ip + u + 1)],
                                identb,
                            )
                        nc.scalar.copy(Gn[:, ip : ip + 4, :], tp2)
                    return ck

                def ck_tps0():
                    Gn = h["Gn"]
                    tps = ps_big.tile([128, D + 2], F32, tag="big")
                    for i in range(NT // 2):
                        nc.tensor.matmul(
                            tps, Gn[:, i, :], c_n[:, i, :],
                            start=(i == 0), stop=False,
                        )
                    h["tps"] = tps

                def ck_tps1():
                    Gn, bq, tps = h["Gn"], h["bq"], h["tps"]
                    for i in range(NT // 2, NT):
                        nc.tensor.matmul(
                            tps, Gn[:, i, :], c_n[:, i, :],
                            start=False, stop=(i == NT - 1),
                        )
                    csi = pb.tile([128, 1], F32, tag="csi")
                    nc.vector.reciprocal(csi, tps[:, D : D + 1])
                    bqc = pb.tile([128, 1], F32, tag="bqc")
                    nc.vector.tensor_mul(bqc, bq, csi)
                    tB = pb.tile([128, D], BF16, tag="tB")
                    nc.vector.tensor_scalar_mul(out=tB, in0=tps[:, 0:D], scalar1=bqc)
                    st["tB"] = tB

                return [
                    ck_qside, ck_qmat, ck_cstore, ck_cT0,
                    mk_cT(0), mk_cT(4), ck_gt0,
                    mk_st(0), mk_st(1), ck_gn0,
                    mk_gn(0), ck_tps0,
                    mk_gn(4), ck_tps1,
                ]

            def out_tile(b, st, i, rsi, pair, pop):
                """c2q/q2c matmuls, normalization, assembly, store: tile i.
                Stores ride in pairs (one descriptor per two tiles).  pop()
                emits at most one pending prep chunk; called at two points so
                no engine queue gets more than one chunk between out ops."""
                c_n, GT, qBx, tB = st["c_n"], st["GT"], st["qBx"], st["tB"]
                gslice = GT[:, 128 * i : 128 * (i + 1)]
                big2 = ps_cq.tile([128, 1024], F32, tag="big2")
                nc.tensor.matmul(
                    big2[:, 0 : D + 2], gslice, qBx, start=True, stop=True
                )
                nc.tensor.matmul(
                    big2[:, 512 : 512 + D], gslice, tB, start=True, stop=True
                )
                nc.vector.reciprocal(rsi[:, i : i + 1], big2[:, D : D + 1])
                pop()
                pop2()

                # ot cols: [c2q | c*c2q | c*q2c] (the c quarter is stored
                # straight from c_n at load time).  Normalize into bf16 SBUF
                # quarters {0,2} first (PSUM read, alternating Act/Pool), then
                # one all-SBUF packed-bf16 DVE mul (runs in 2x/4x DVE mode)
                # fills the c-product quarters {1,2}.
                if i % 2 == 0:
                    pair["ot"] = po.tile([128, 2, 3 * D], BF16, tag="ot", name="ot")
                ot3 = pair["ot"][:, i % 2, :].rearrange("p (j x) -> p j x", x=D)
                big_v = big2.rearrange("p (j x) -> p j x", j=2)[:, :, 0:D]
                if i % 2 == 0:
                    nc.scalar.mul(ot3[:, 0:3:2, :], big_v, rsi[:, i : i + 1])
                else:
                    nc.vector.tensor_scalar_mul(
                        out=ot3[:, 0:3:2, :], in0=big_v, scalar1=rsi[:, i : i + 1]
                    )
                peng = nc.vector if i % 2 == 0 else nc.gpsimd
                peng.tensor_mul(
                    ot3[:, 1:3, :],
                    ot3[:, 0:3:2, :],
                    c_n[:, i, 0:D].unsqueeze(1).to_broadcast([128, 2, D]),
                )
                if i % 2 == 1:
                    nc.sync.dma_start(
                        out=out_d[b, 128 * (i - 1) : 128 * (i + 1), D : 4 * D]
                        .rearrange("(k p) x -> p k x", p=128),
                        in_=pair["ot"],
                    )
                pop()

            # software pipeline, two batches deep: prep(b+1) is fully emitted
            # before out(b) begins (it was interleaved into out(b-1)), so the
            # seam between batches never waits on prep latency.  During
            # out(b) we emit loads(b+2) [DMA queue only] and interleave the
            # prep chunks of b+2 with the out tiles.
            sts = [None] * (bpc + 2)
            sts[0] = load_stage(0)
            for ck in prep_chunks(0, sts[0]):
                ck()
            if bpc > 1:
                sts[1] = load_stage(1)
                for ck in prep_chunks(1, sts[1]):
                    ck()
            from collections import deque

            pending = deque()

            def pop():
                if pending:
                    pending.popleft()()

            def pop2():
                # extra drain slot, used when the queue is running long so
                # prep never falls more than a batch behind
                if len(pending) > 8:
                    pending.popleft()()

            for b in range(bpc):
                if b + 2 < bpc:
                    sts[b + 2] = load_stage(b + 2)
                    pending.extend(prep_chunks(b + 2, sts[b + 2]))
                rsi = pb.tile([128, NT], F32, tag="rsi")
                pair = {}
                for i in range(NT):
                    out_tile(b, sts[b], i, rsi, pair, pop)
            while pending:
                pending.popleft()()

    nc.finalize()
    return nc


_NC = None


def _get_nc():
    global _NC
    if _NC is None:
        _NC = _build()
    return _NC


def kernel(c, q, c_mask, q_mask, w0, w1, wm, bias):
    from ml_dtypes import bfloat16
    c = np.ascontiguousarray(np.asarray(c, dtype=np.float32).astype(bfloat16))
    q = np.ascontiguousarray(np.asarray(q, dtype=np.float32).astype(bfloat16))
    cmb = np.ascontiguousarray(
        30.0 * (c_mask.astype(np.float32) - 1.0), dtype=np.float32
    )
    q_mask = np.ascontiguousarray(q_mask, dtype=np.int32)
    w0 = np.ascontiguousarray(w0, dtype=np.float32)
    w1 = np.ascontiguousarray(w1, dtype=np.float32)
    wm = np.ascontiguousarray(wm, dtype=np.float32)
    bias = np.ascontiguousarray(bias, dtype=np.float32)

    in_maps = []
    for k in range(NCORES):
        s = slice(k * BPC, (k + 1) * BPC)
        in_maps.append(
            {
                "c": c[s],
                "q": q[s],
                "cmb": cmb[s],
                "q_mask": q_mask[s],
                "w0": w0,
                "w1": w1,
                "wm": wm,
                "bias": bias,
            }
        )

    res = run_bass_kernel_spmd(_get_nc(), in_maps, core_ids=list(range(NCORES)))
    return np.concatenate(
        [res.results[k]["out"].astype(np.float32) for k in range(NCORES)], axis=0
    )
